# revision 12
# baseline (speedup 1.0000x reference)
"""HNHN hypergraph GNN forward on 8 Trainium2 NeuronCores (Bass/Tile).

Sharding: edges 50k/core, nodes 25k/core (edge ids relabeled e -> (e%8, e//8)
for load balance; relabeling is internal, the output is a node max-pool).
Each segment aggregation is computed as PE matmuls
    psum[feat, 512segs] += G_block^T @ S_block
with G_block = 128 gathered bf16 source rows and S_block a one-hot x weight
selection matrix built on DVE via tensor_scalar(is_equal, mult) against an
iota tile. Layer-1 edge aggregation consumes a host-expanded per-nnz stream
of x_0 (input resharding; no gather). The other three aggregations gather
device-computed bf16 tables with dma_gather (1024 rows/instruction, int16
indices => 32k-row buckets; nnz sorted by (psum-section, bucket, seg); runs
padded with trailing -1 indices which the DMA skips). The per-layer dense
matmul, sigmoid (+per-partition bias) happen in the transposed [feat, seg]
domain; PE transposes restore row-major bf16 tables which are AllGather'd
for the next aggregation. Final: running window max, AllReduce(max), f32 dot
with lin_w.
"""

import numpy as np
import ml_dtypes

bf16 = ml_dtypes.bfloat16
f32 = np.float32

P = 128
WIN = 512          # segments per PSUM window (one bank)
SECW = 5           # windows per section (PSUM: 5 win + 1 tp + 2 m = 8 banks)
BUCK = 32768       # dma_gather int16 index range
NIMAX = 1024       # dma_gather max idxs/instruction (ring capacity)
NCORES = 8

N_NODES = 200_000
N_EDGES = 400_000
IN_CH = 14
HID = 128
ALPHA = -1.5
BETA = -0.5


def _dims():
    node_loc = N_NODES // NCORES
    edge_loc = N_EDGES // NCORES
    node_wins = -(-node_loc // WIN)
    edge_wins = -(-edge_loc // WIN)
    return dict(
        node_loc=node_loc, edge_loc=edge_loc,
        node_pad=node_wins * WIN, edge_pad=edge_wins * WIN,
        node_tab=node_wins * WIN * NCORES, edge_tab=edge_wins * WIN * NCORES,
    )


def _wrap_idx(flat):
    a = flat.reshape(-1, 16).T.astype(np.int16)
    return np.tile(a, (8, 1))


class AggSched:
    """Schedule + per-core metadata for one aggregation (SPMD-identical)."""

    def __init__(self, name, dest_loc, src_all, w_all, n_seg_loc, tab_rows,
                 gathered):
        self.name = name
        self.gathered = gathered
        self.n_seg_loc = n_seg_loc
        self.n_wins = -(-n_seg_loc // WIN)
        sec = WIN * SECW
        self.n_secs = -(-self.n_wins // SECW)
        nbuck = -(-tab_rows // BUCK) if gathered else 1

        per_core = []
        counts = np.zeros((NCORES, self.n_secs, nbuck), np.int64)
        for r in range(NCORES):
            d = dest_loc[r].astype(np.int64)
            s = src_all[r].astype(np.int64)
            w = w_all[r].astype(f32)
            sc = d // sec
            b = (s // BUCK) if gathered else np.zeros_like(s)
            order = np.lexsort((d, b, sc))
            d, s, w, sc, b = d[order], s[order], w[order], sc[order], b[order]
            per_core.append((d, s, w, sc, b))
            np.add.at(counts[r], (sc, b), 1)

        self.runs = []          # (sec, bucket, n_pad_slots)
        for sc in range(self.n_secs):
            for b in range(nbuck):
                c = counts[:, sc, b].max()
                if c:
                    self.runs.append((sc, b, int(-(-c // 128) * 128)))
        total_slots = sum(np_ for _, _, np_ in self.runs)
        self.n_blocks = total_slots // 128

        dmat = np.full((NCORES, total_slots), -1, np.int64)
        smat = np.full((NCORES, total_slots), -1, np.int64)
        wmat = np.zeros((NCORES, total_slots), f32)
        for r in range(NCORES):
            d, s, w, sc, b = per_core[r]
            off = 0
            ptr = 0
            for rsec, rb, n_pad in self.runs:
                cnt = int(counts[r, rsec, rb])
                dmat[r, off:off + cnt] = d[ptr:ptr + cnt]
                smat[r, off:off + cnt] = (s[ptr:ptr + cnt] % BUCK) if gathered \
                    else s[ptr:ptr + cnt]
                if gathered:
                    smat[r, off + cnt:off + n_pad] = 0  # pad -> bucket row 0
                wmat[r, off:off + cnt] = w[ptr:ptr + cnt]
                ptr += cnt
                off += n_pad
            assert ptr == len(d)

        self.insts = []         # (bucket, slot_off, ni)
        off = 0
        for sc, b, n_pad in self.runs:
            o = 0
            while o < n_pad:
                ni = min(NIMAX, n_pad - o)
                self.insts.append((b, off + o, ni))
                o += ni
            off += n_pad

        dblk = dmat.reshape(NCORES, self.n_blocks, 128)
        self.jobs = []          # (block, window, c0, c1)
        for blk in range(self.n_blocks):
            dv = dblk[:, blk, :]
            valid = dv >= 0
            if not valid.any():
                continue
            for wname in np.unique(dv[valid] // WIN):
                m = valid & (dv // WIN == wname)
                c0 = int((dv[m] - wname * WIN).min())
                c1 = int((dv[m] - wname * WIN).max()) + 1
                self.jobs.append((blk, int(wname), c0, c1))
        self.n_jobs = len(self.jobs)
        self.win_last = {}
        for j, (blk, wname, c0, c1) in enumerate(self.jobs):
            self.win_last[wname] = j

        self.w_arr = np.ascontiguousarray(
            wmat.reshape(NCORES, self.n_blocks, 128).transpose(0, 2, 1))
        self.seg_arr = np.full((NCORES, P, max(self.n_jobs, 1)), -1.0, f32)
        for j, (blk, wname, c0, c1) in enumerate(self.jobs):
            dv = dblk[:, blk, :]                    # [NCORES, 128]
            ok = (dv >= 0) & (dv // WIN == wname)
            self.seg_arr[:, :, j] = np.where(ok, dv - wname * WIN, -1.0)
        self.seg_arr = self.seg_arr.astype(f32)

        if gathered:
            idx_cols = sum(ni for _, _, ni in self.insts) // 16
            self.idx_arr = np.zeros((NCORES, P, idx_cols), np.int16)
            self.inst_idx_off = []
            col = 0
            for b, slot_off, ni in self.insts:
                self.inst_idx_off.append(col)
                for r in range(NCORES):
                    self.idx_arr[r, :, col:col + ni // 16] = _wrap_idx(
                        smat[r, slot_off:slot_off + ni].astype(np.int16))
                col += ni // 16
            self.idx_cols = idx_cols
        else:
            self.smat = smat


def _preprocess(inputs):
    dims = _dims()
    rows = np.asarray(inputs["inc_rows"]).astype(np.int64)
    cols0 = np.asarray(inputs["inc_cols"]).astype(np.int64)
    vals = np.asarray(inputs["inc_vals"]).astype(f32)

    # relabel edges for per-core balance: e -> (e % NCORES)*edge_loc + e//NCORES
    cols = (cols0 % NCORES) * dims["edge_loc"] + cols0 // NCORES

    deg_e = np.bincount(cols, weights=vals, minlength=N_EDGES).astype(f32)
    deg_v = np.bincount(rows, weights=vals, minlength=N_NODES).astype(f32)
    e_card = deg_e ** f32(ALPHA)
    n_card = deg_v ** f32(BETA)
    denom_v = np.bincount(rows, weights=(vals * e_card[cols]).astype(np.float64),
                          minlength=N_NODES).astype(f32)
    denom_e = np.bincount(cols, weights=(vals * n_card[rows]).astype(np.float64),
                          minlength=N_EDGES).astype(f32)
    w_ev = vals * n_card[rows] / denom_e[cols]
    w_ve = vals * e_card[cols] / denom_v[rows]

    e_core = cols // dims["edge_loc"]
    v_core = rows // dims["node_loc"]
    node_pad_row = rows // dims["node_loc"] * dims["node_pad"] \
        + rows % dims["node_loc"]
    edge_pad_row = cols // dims["edge_loc"] * dims["edge_pad"] \
        + cols % dims["edge_loc"]

    def split(arr, by):
        return [arr[by == r] for r in range(NCORES)]

    e_d = split(cols % dims["edge_loc"], e_core)
    e_s_raw = split(rows, e_core)
    e_s_pad = split(node_pad_row, e_core)
    e_w = split(w_ev, e_core)
    n_d = split(rows % dims["node_loc"], v_core)
    n_s = split(edge_pad_row, v_core)
    n_w = split(w_ve, v_core)

    sched_e1 = AggSched("e1", e_d, e_s_raw, e_w, dims["edge_loc"], N_NODES,
                        False)
    sched_e2 = AggSched("e2", e_d, e_s_pad, e_w, dims["edge_loc"],
                        dims["node_tab"], True)
    sched_n = AggSched("n", n_d, n_s, n_w, dims["node_loc"],
                       dims["edge_tab"], True)

    x0 = np.asarray(inputs["x_0"]).astype(f32)
    x0p = np.zeros((N_NODES + 1, 16), f32)
    x0p[:N_NODES, :IN_CH] = x0
    e1_stream = np.zeros((NCORES, P, sched_e1.n_blocks * 16), bf16)
    for r in range(NCORES):
        src = sched_e1.smat[r].reshape(sched_e1.n_blocks, 128)
        g = x0p[np.where(src >= 0, src, N_NODES)]
        e1_stream[r] = g.transpose(1, 0, 2).reshape(P, -1).astype(bf16)

    return dict(sched_e1=sched_e1, sched_e2=sched_e2, sched_n=sched_n,
                e1_stream=e1_stream, dims=dims)


def _build(pre):
    import concourse.bacc as bacc
    import concourse.mybir as mybir
    import concourse.tile as tile

    dt = mybir.dt
    dims = pre["dims"]
    nc = bacc.Bacc("TRN2", target_bir_lowering=False, debug=False,
                   num_devices=NCORES)

    s_e1, s_e2, s_n = pre["sched_e1"], pre["sched_e2"], pre["sched_n"]

    def din(name, shape, dtyp):
        return nc.dram_tensor(name, shape, dtyp, kind="ExternalInput")

    e1_g = din("e1_g", [P, s_e1.n_blocks * 16], dt.bfloat16)
    e1_seg = din("e1_seg", [P, max(s_e1.n_jobs, 1)], dt.float32)
    e1_w = din("e1_w", [P, s_e1.n_blocks], dt.float32)
    n1_idx = din("n1_idx", [P, s_n.idx_cols], dt.int16)
    n1_seg = din("n1_seg", [P, max(s_n.n_jobs, 1)], dt.float32)
    n1_w = din("n1_w", [P, s_n.n_blocks], dt.float32)
    e2_idx = din("e2_idx", [P, s_e2.idx_cols], dt.int16)
    e2_seg = din("e2_seg", [P, max(s_e2.n_jobs, 1)], dt.float32)
    e2_w = din("e2_w", [P, s_e2.n_blocks], dt.float32)

    w_in = {k: din(k, [kd, HID], dt.bfloat16)
            for k, kd in (("w0_1", 16), ("w1_1", HID), ("w0_2", HID),
                          ("w1_2", HID))}
    b_in = {k: din(k, [P, 1], dt.float32)
            for k in ("b1_1", "b0_1", "b1_2", "b0_2")}
    lin_w = din("lin_w", [P, 1], dt.float32)
    lin_b = din("lin_b", [1, 1], dt.float32)
    iota_in = din("iota", [P, WIN], dt.float32)
    ident_in = din("ident", [P, P], dt.bfloat16)
    out_t = nc.dram_tensor("out", [1, 1], dt.float32, kind="ExternalOutput")

    def dint(name, shape, shared=False):
        return nc.dram_tensor(name, shape, dt.bfloat16, kind="Internal",
                              addr_space="Shared" if shared else "Local")

    x1l1_loc = dint("x1l1_loc", [dims["edge_pad"], HID])
    x1l1_full = dint("x1l1_full", [dims["edge_tab"], HID], True)
    x0p_loc = dint("x0p_loc", [dims["node_pad"], HID])
    x0p_full = dint("x0p_full", [dims["node_tab"], HID], True)
    x1l2_loc = dint("x1l2_loc", [dims["edge_pad"], HID])
    x1l2_full = dint("x1l2_full", [dims["edge_tab"], HID], True)
    armax_in = nc.dram_tensor("armax_in", [P, 1], dt.float32, kind="Internal")
    armax_out = nc.dram_tensor("armax_out", [P, 1], dt.float32,
                               kind="Internal", addr_space="Shared")

    import os as _os0
    GP_BUFS = int(_os0.environ.get("GP_BUFS", "6"))
    SP_BUFS = int(_os0.environ.get("SP_BUFS", "4"))
    with tile.TileContext(nc) as tc:
        with tc.tile_pool(name="const", bufs=1) as cp, \
             tc.tile_pool(name="meta", bufs=int(_os0.environ.get("MP_BUFS", "2"))) as mp, \
             tc.tile_pool(name="gt", bufs=GP_BUFS) as gp, \
             tc.tile_pool(name="st", bufs=SP_BUFS) as sp, \
             tc.tile_pool(name="fl", bufs=2) as fp, \
             tc.tile_pool(name="psw", bufs=1, space="PSUM") as pw, \
             tc.tile_pool(name="psm", bufs=2, space="PSUM") as pm:

            iota_t = cp.tile([P, WIN], dt.float32)
            ident_t = cp.tile([P, P], dt.bfloat16)
            nc.sync.dma_start(iota_t[:], iota_in[:])
            nc.sync.dma_start(ident_t[:], ident_in[:])
            wts, bias = {}, {}
            for k, hnd in w_in.items():
                t = cp.tile(list(hnd.shape), dt.bfloat16, tag=k)
                nc.sync.dma_start(t[:], hnd[:])
                wts[k] = t
            for k, hnd in b_in.items():
                t = cp.tile([P, 1], dt.float32, tag=k)
                nc.sync.dma_start(t[:], hnd[:])
                bias[k] = t
            linw_t = cp.tile([P, 1], dt.float32)
            nc.sync.dma_start(linw_t[:], lin_w[:])
            linb_t = cp.tile([1, 1], dt.float32)
            nc.sync.dma_start(linb_t[:], lin_b[:])
            maxacc = cp.tile([P, WIN], dt.bfloat16)
            nc.vector.memset(maxacc[:], -1.0)

            def run_agg(sched, seg_d, w_d, kdim, wkey, bkey, table, out_loc,
                        idx_d=None, stream_d=None, maxpool=False):
                import os as _os
                LHSW = int(_os.environ.get("LHSW", "0"))
                if LHSW and sched.gathered:
                    kdim = LHSW
                seg_t = mp.tile([P, max(sched.n_jobs, 1)], dt.float32,
                                tag="seg")
                w_t = mp.tile([P, sched.n_blocks], dt.float32, tag="w")
                nc.sync.dma_start(seg_t[:], seg_d[:])
                nc.sync.dma_start(w_t[:], w_d[:])

                import os as _os
                NOGATH = int(_os.environ.get("NOGATH", "0"))
                NOJOBS = int(_os.environ.get("NOJOBS", "0"))
                blk_slice = {}
                if sched.gathered:
                    idx_t = mp.tile([P, sched.idx_cols], dt.int16, tag="idx")
                    nc.sync.dma_start(idx_t[:], idx_d[:])
                    tab_rows = table.shape[0]
                    for k, (b, slot_off, ni) in enumerate(sched.insts):
                        g = gp.tile([P, (NIMAX // P) * HID], dt.bfloat16,
                                    tag="g")
                        off = sched.inst_idx_off[k]
                        if not NOGATH:
                            nc.gpsimd.dma_gather(
                                g[:, :(ni // P) * HID].rearrange(
                                    "p (n f) -> p n f", f=HID),
                                table[b * BUCK:min((b + 1) * BUCK, tab_rows), :],
                                idx_t[:, off:off + ni // 16],
                                ni, ni, HID)
                        else:
                            nc.vector.memset(g[:1, :1], 0.0)
                        for cb in range(ni // P):
                            blk_slice[slot_off // P + cb] = (g, cb * HID, HID)
                else:
                    SLAB = 32
                    for sl in range(-(-sched.n_blocks // SLAB)):
                        b0 = sl * SLAB
                        nb = min(SLAB, sched.n_blocks - b0)
                        g = gp.tile([P, SLAB * 16], dt.bfloat16, tag="g")
                        nc.sync.dma_start(g[:, :nb * 16],
                                          stream_d[:, b0 * 16:(b0 + nb) * 16])
                        for cb in range(nb):
                            blk_slice[b0 + cb] = (g, cb * 16, 16)

                win_tiles = {}

                def flush(wn):
                    psum1 = win_tiles.pop(wn)
                    aggt = fp.tile([kdim, WIN], dt.bfloat16, tag="aggt")
                    nc.vector.tensor_copy(aggt[:], psum1[:])
                    psum2 = pm.tile([P, WIN], dt.float32, tag="m",
                                    space="PSUM")
                    nc.tensor.matmul(psum2[:], lhsT=wts[wkey][:kdim, :],
                                     rhs=aggt[:], start=True, stop=True)
                    xt = fp.tile([P, WIN], dt.bfloat16, tag="xt")
                    nc.scalar.activation(xt[:], psum2[:],
                                         mybir.ActivationFunctionType.Sigmoid,
                                         bias=bias[bkey][:, :1], scale=1.0)
                    if maxpool:
                        nv = min(WIN, sched.n_seg_loc - wn * WIN)
                        nc.vector.tensor_tensor(
                            out=maxacc[:, :nv], in0=maxacc[:, :nv],
                            in1=xt[:, :nv], op=mybir.AluOpType.max)
                    else:
                        nq = WIN // P
                        rowt = fp.tile([P, WIN], dt.bfloat16, tag="rowt")
                        for q in range(nq):
                            pt = pw.tile([P, P], dt.bfloat16, tag="tp",
                                         space="PSUM")
                            nc.tensor.transpose(pt[:],
                                                xt[:, q * P:(q + 1) * P],
                                                ident_t[:])
                            nc.vector.tensor_copy(rowt[:, q * P:(q + 1) * P],
                                                  pt[:])
                        nc.sync.dma_start(
                            out_loc[wn * WIN:(wn + 1) * WIN, :].rearrange(
                                "(q p) f -> p q f", p=P),
                            rowt[:].rearrange("p (q f) -> p q f", q=nq))

                TS_MEMSET = int(_os.environ.get("TS_MEMSET", "0"))
                MM_FIXED = int(_os.environ.get("MM_FIXED", "0"))
                MM_SS = int(_os.environ.get("MM_SS", "0"))
                variant = sched.gathered
                for j, (blk, wn, c0, c1) in enumerate(sched.jobs):
                    if NOJOBS:
                        break
                    g, goff, gw = blk_slice[blk]
                    span = c1 - c0
                    s_t = sp.tile([P, WIN], dt.bfloat16, tag="s")
                    if variant and TS_MEMSET:
                        nc.vector.memset(s_t[:, :span], 0.5)
                    else:
                        nc.vector.tensor_scalar(
                            out=s_t[:, :span], in0=iota_t[:, c0:c1],
                            scalar1=seg_t[:, j:j + 1],
                            scalar2=w_t[:, blk:blk + 1],
                            op0=mybir.AluOpType.is_equal,
                            op1=mybir.AluOpType.mult)
                    if wn not in win_tiles:
                        pt = pw.tile([kdim, WIN], dt.float32,
                                     tag=f"win{wn % SECW}", space="PSUM")
                        nc.vector.memset(pt[:], 0.0)
                        win_tiles[wn] = pt
                    mmc0, mmc1 = (0, span) if (variant and MM_FIXED) else (c0, c1)
                    if variant and MM_SS:
                        nc.tensor.matmul(
                            win_tiles[wn][:, mmc0:mmc1],
                            lhsT=g[:, goff:goff + kdim],
                            rhs=s_t[:, :span], start=True, stop=True,
                            skip_group_check=True)
                    else:
                        nc.tensor.matmul(
                            win_tiles[wn][:, mmc0:mmc1],
                            lhsT=g[:, goff:goff + kdim],
                            rhs=s_t[:, :span], start=False,
                            stop=(sched.win_last[wn] == j),
                            skip_group_check=True)
                    if sched.win_last[wn] == j:
                        flush(wn)

            import os
            PH = int(os.environ.get("PHASES", "4"))
            NOCOLL = int(os.environ.get("NOCOLL", "0"))
            rg = [list(range(NCORES))]
            if PH >= 1:
                run_agg(s_e1, e1_seg, e1_w, 16, "w0_1", "b1_1", None, x1l1_loc,
                        stream_d=e1_g)
            if PH >= 2:
                if not NOCOLL:
                    nc.gpsimd.collective_compute(
                        "AllGather", mybir.AluOpType.bypass, replica_groups=rg,
                        ins=[x1l1_loc[:]], outs=[x1l1_full[:]])
                run_agg(s_n, n1_seg, n1_w, HID, "w1_1", "b0_1", x1l1_full,
                        x0p_loc, idx_d=n1_idx)
            if PH >= 3:
                if not NOCOLL:
                    nc.gpsimd.collective_compute(
                        "AllGather", mybir.AluOpType.bypass, replica_groups=rg,
                        ins=[x0p_loc[:]], outs=[x0p_full[:]])
                run_agg(s_e2, e2_seg, e2_w, HID, "w0_2", "b1_2", x0p_full,
                        x1l2_loc, idx_d=e2_idx)
            if PH >= 4:
                if not NOCOLL:
                    nc.gpsimd.collective_compute(
                        "AllGather", mybir.AluOpType.bypass, replica_groups=rg,
                        ins=[x1l2_loc[:]], outs=[x1l2_full[:]])
                run_agg(s_n, n1_seg, n1_w, HID, "w1_2", "b0_2", x1l2_full,
                        None, idx_d=n1_idx, maxpool=True)

            mx = fp.tile([P, 1], dt.float32, tag="mx")
            nc.vector.reduce_max(out=mx[:], in_=maxacc[:],
                                 axis=mybir.AxisListType.X)
            nc.sync.dma_start(armax_in[:], mx[:])
            nc.gpsimd.collective_compute(
                "AllReduce", mybir.AluOpType.max, replica_groups=rg,
                ins=[armax_in[:]], outs=[armax_out[:]])
            mx2 = fp.tile([P, 1], dt.float32, tag="mx2")
            nc.sync.dma_start(mx2[:], armax_out[:])
            prod = fp.tile([P, 1], dt.float32, tag="prod")
            nc.vector.tensor_mul(prod[:], mx2[:], linw_t[:])
            ones = cp.tile([P, 1], dt.float32, tag="ones")
            nc.vector.memset(ones[:], 1.0)
            psf = pw.tile([1, 1], dt.float32, tag="tp", space="PSUM")
            nc.tensor.matmul(psf[:], lhsT=prod[:], rhs=ones[:],
                             start=True, stop=True)
            res = fp.tile([1, 1], dt.float32, tag="res")
            nc.scalar.activation(res[:], psf[:],
                                 mybir.ActivationFunctionType.Identity,
                                 bias=linb_t[:, :1], scale=1.0)
            nc.sync.dma_start(out_t[:], res[:])

    nc.compile()
    return nc


def make_in_maps(pre, inputs):
    s_e1, s_e2, s_n = pre["sched_e1"], pre["sched_e2"], pre["sched_n"]
    iota = np.broadcast_to(np.arange(WIN, dtype=f32), (P, WIN)).copy()
    ident = np.eye(P, dtype=bf16)

    def b_t(x):
        return np.asarray(x).astype(f32).reshape(HID, 1)

    w0_1 = np.zeros((16, HID), bf16)
    w0_1[:IN_CH] = np.asarray(inputs["w0_l1"]).astype(bf16)
    in_maps = []
    for r in range(NCORES):
        in_maps.append(dict(
            e1_g=pre["e1_stream"][r],
            e1_seg=np.ascontiguousarray(s_e1.seg_arr[r]),
            e1_w=np.ascontiguousarray(s_e1.w_arr[r]),
            n1_idx=np.ascontiguousarray(s_n.idx_arr[r]),
            n1_seg=np.ascontiguousarray(s_n.seg_arr[r]),
            n1_w=np.ascontiguousarray(s_n.w_arr[r]),
            e2_idx=np.ascontiguousarray(s_e2.idx_arr[r]),
            e2_seg=np.ascontiguousarray(s_e2.seg_arr[r]),
            e2_w=np.ascontiguousarray(s_e2.w_arr[r]),
            w0_1=w0_1,
            w1_1=np.asarray(inputs["w1_l1"]).astype(bf16),
            w0_2=np.asarray(inputs["w0_l2"]).astype(bf16),
            w1_2=np.asarray(inputs["w1_l2"]).astype(bf16),
            b1_1=b_t(inputs["b1_l1"]), b0_1=b_t(inputs["b0_l1"]),
            b1_2=b_t(inputs["b1_l2"]), b0_2=b_t(inputs["b0_l2"]),
            lin_w=np.asarray(inputs["lin_w"]).astype(f32).reshape(HID, 1),
            lin_b=np.asarray(inputs["lin_b"]).astype(f32).reshape(1, 1),
            iota=iota, ident=ident,
        ))
    return in_maps


def kernel(**inputs):
    pre = _preprocess(inputs)
    nc = _build(pre)
    in_maps = make_in_maps(pre, inputs)
    from concourse.bass_utils import run_bass_kernel_spmd
    res = run_bass_kernel_spmd(nc, in_maps, core_ids=list(range(NCORES)))
    out = res.results[0]["out"].reshape(1).astype(f32)
    return out



# revision 13
# speedup vs baseline: 1.0028x; 1.0028x over previous
"""HNHN hypergraph GNN forward on 8 Trainium2 NeuronCores (Bass/Tile).

Sharding: edges 50k/core, nodes 25k/core (edge ids relabeled e -> (e%8, e//8)
for load balance; relabeling is internal, the output is a node max-pool).
Each segment aggregation is computed as PE matmuls
    psum[feat, 512segs] += G_block^T @ S_block
with G_block = 128 gathered bf16 source rows and S_block a one-hot x weight
selection matrix built on DVE via tensor_scalar(is_equal, mult) against an
iota tile. Layer-1 edge aggregation consumes a host-expanded per-nnz stream
of x_0 (input resharding; no gather). The other three aggregations gather
device-computed bf16 tables with dma_gather (1024 rows/instruction, int16
indices => 32k-row buckets; nnz sorted by (psum-section, bucket, seg); runs
padded with trailing -1 indices which the DMA skips). The per-layer dense
matmul, sigmoid (+per-partition bias) happen in the transposed [feat, seg]
domain; PE transposes restore row-major bf16 tables which are AllGather'd
for the next aggregation. Final: running window max, AllReduce(max), f32 dot
with lin_w.
"""

import numpy as np
import ml_dtypes

bf16 = ml_dtypes.bfloat16
f32 = np.float32

P = 128
WIN = 512          # segments per PSUM window (one bank)
SECW = 5           # windows per section (PSUM: 5 win + 1 tp + 2 m = 8 banks)
BUCK = 32768       # dma_gather int16 index range
NIMAX = 1024       # dma_gather max idxs/instruction (ring capacity)
NCORES = 8

N_NODES = 200_000
N_EDGES = 400_000
IN_CH = 14
HID = 128
ALPHA = -1.5
BETA = -0.5


def _dims():
    node_loc = N_NODES // NCORES
    edge_loc = N_EDGES // NCORES
    node_wins = -(-node_loc // WIN)
    edge_wins = -(-edge_loc // WIN)
    return dict(
        node_loc=node_loc, edge_loc=edge_loc,
        node_pad=node_wins * WIN, edge_pad=edge_wins * WIN,
        node_tab=node_wins * WIN * NCORES, edge_tab=edge_wins * WIN * NCORES,
    )


def _wrap_idx(flat):
    a = flat.reshape(-1, 16).T.astype(np.int16)
    return np.tile(a, (8, 1))


class AggSched:
    """Schedule + per-core metadata for one aggregation (SPMD-identical)."""

    def __init__(self, name, dest_loc, src_all, w_all, n_seg_loc, tab_rows,
                 gathered):
        self.name = name
        self.gathered = gathered
        self.n_seg_loc = n_seg_loc
        self.n_wins = -(-n_seg_loc // WIN)
        sec = WIN * SECW
        self.n_secs = -(-self.n_wins // SECW)
        nbuck = -(-tab_rows // BUCK) if gathered else 1

        per_core = []
        counts = np.zeros((NCORES, self.n_secs, nbuck), np.int64)
        for r in range(NCORES):
            d = dest_loc[r].astype(np.int64)
            s = src_all[r].astype(np.int64)
            w = w_all[r].astype(f32)
            sc = d // sec
            b = (s // BUCK) if gathered else np.zeros_like(s)
            order = np.lexsort((d, b, sc))
            d, s, w, sc, b = d[order], s[order], w[order], sc[order], b[order]
            per_core.append((d, s, w, sc, b))
            np.add.at(counts[r], (sc, b), 1)

        self.runs = []          # (sec, bucket, n_pad_slots)
        for sc in range(self.n_secs):
            for b in range(nbuck):
                c = counts[:, sc, b].max()
                if c:
                    self.runs.append((sc, b, int(-(-c // 128) * 128)))
        total_slots = sum(np_ for _, _, np_ in self.runs)
        self.n_blocks = total_slots // 128

        dmat = np.full((NCORES, total_slots), -1, np.int64)
        smat = np.full((NCORES, total_slots), -1, np.int64)
        wmat = np.zeros((NCORES, total_slots), f32)
        for r in range(NCORES):
            d, s, w, sc, b = per_core[r]
            off = 0
            ptr = 0
            for rsec, rb, n_pad in self.runs:
                cnt = int(counts[r, rsec, rb])
                dmat[r, off:off + cnt] = d[ptr:ptr + cnt]
                smat[r, off:off + cnt] = (s[ptr:ptr + cnt] % BUCK) if gathered \
                    else s[ptr:ptr + cnt]
                if gathered:
                    smat[r, off + cnt:off + n_pad] = 0  # pad -> bucket row 0
                wmat[r, off:off + cnt] = w[ptr:ptr + cnt]
                ptr += cnt
                off += n_pad
            assert ptr == len(d)

        self.insts = []         # (bucket, slot_off, ni)
        off = 0
        for sc, b, n_pad in self.runs:
            o = 0
            while o < n_pad:
                ni = min(NIMAX, n_pad - o)
                self.insts.append((b, off + o, ni))
                o += ni
            off += n_pad

        dblk = dmat.reshape(NCORES, self.n_blocks, 128)
        self.jobs = []          # (block, window, c0, c1)
        for blk in range(self.n_blocks):
            dv = dblk[:, blk, :]
            valid = dv >= 0
            if not valid.any():
                continue
            for wname in np.unique(dv[valid] // WIN):
                m = valid & (dv // WIN == wname)
                c0 = int((dv[m] - wname * WIN).min())
                c1 = int((dv[m] - wname * WIN).max()) + 1
                self.jobs.append((blk, int(wname), c0, c1))
        self.n_jobs = len(self.jobs)
        self.win_last = {}
        for j, (blk, wname, c0, c1) in enumerate(self.jobs):
            self.win_last[wname] = j

        self.w_arr = np.ascontiguousarray(
            wmat.reshape(NCORES, self.n_blocks, 128).transpose(0, 2, 1))
        self.seg_arr = np.full((NCORES, P, max(self.n_jobs, 1)), -1.0, f32)
        for j, (blk, wname, c0, c1) in enumerate(self.jobs):
            dv = dblk[:, blk, :]                    # [NCORES, 128]
            ok = (dv >= 0) & (dv // WIN == wname)
            self.seg_arr[:, :, j] = np.where(ok, dv - wname * WIN, -1.0)
        self.seg_arr = self.seg_arr.astype(f32)

        if gathered:
            idx_cols = sum(ni for _, _, ni in self.insts) // 16
            self.idx_arr = np.zeros((NCORES, P, idx_cols), np.int16)
            self.inst_idx_off = []
            col = 0
            for b, slot_off, ni in self.insts:
                self.inst_idx_off.append(col)
                for r in range(NCORES):
                    self.idx_arr[r, :, col:col + ni // 16] = _wrap_idx(
                        smat[r, slot_off:slot_off + ni].astype(np.int16))
                col += ni // 16
            self.idx_cols = idx_cols
        else:
            self.smat = smat


def _preprocess(inputs):
    dims = _dims()
    rows = np.asarray(inputs["inc_rows"]).astype(np.int64)
    cols0 = np.asarray(inputs["inc_cols"]).astype(np.int64)
    vals = np.asarray(inputs["inc_vals"]).astype(f32)

    # relabel edges for per-core balance: e -> (e % NCORES)*edge_loc + e//NCORES
    cols = (cols0 % NCORES) * dims["edge_loc"] + cols0 // NCORES

    deg_e = np.bincount(cols, weights=vals, minlength=N_EDGES).astype(f32)
    deg_v = np.bincount(rows, weights=vals, minlength=N_NODES).astype(f32)
    e_card = deg_e ** f32(ALPHA)
    n_card = deg_v ** f32(BETA)
    denom_v = np.bincount(rows, weights=(vals * e_card[cols]).astype(np.float64),
                          minlength=N_NODES).astype(f32)
    denom_e = np.bincount(cols, weights=(vals * n_card[rows]).astype(np.float64),
                          minlength=N_EDGES).astype(f32)
    w_ev = vals * n_card[rows] / denom_e[cols]
    w_ve = vals * e_card[cols] / denom_v[rows]

    e_core = cols // dims["edge_loc"]
    v_core = rows // dims["node_loc"]
    node_pad_row = rows // dims["node_loc"] * dims["node_pad"] \
        + rows % dims["node_loc"]
    edge_pad_row = cols // dims["edge_loc"] * dims["edge_pad"] \
        + cols % dims["edge_loc"]

    def split(arr, by):
        return [arr[by == r] for r in range(NCORES)]

    e_d = split(cols % dims["edge_loc"], e_core)
    e_s_raw = split(rows, e_core)
    e_s_pad = split(node_pad_row, e_core)
    e_w = split(w_ev, e_core)
    n_d = split(rows % dims["node_loc"], v_core)
    n_s = split(edge_pad_row, v_core)
    n_w = split(w_ve, v_core)

    sched_e1 = AggSched("e1", e_d, e_s_raw, e_w, dims["edge_loc"], N_NODES,
                        False)
    sched_e2 = AggSched("e2", e_d, e_s_pad, e_w, dims["edge_loc"],
                        dims["node_tab"], True)
    sched_n = AggSched("n", n_d, n_s, n_w, dims["node_loc"],
                       dims["edge_tab"], True)

    x0 = np.asarray(inputs["x_0"]).astype(f32)
    x0p = np.zeros((N_NODES + 1, 16), f32)
    x0p[:N_NODES, :IN_CH] = x0
    e1_stream = np.zeros((NCORES, P, sched_e1.n_blocks * 16), bf16)
    for r in range(NCORES):
        src = sched_e1.smat[r].reshape(sched_e1.n_blocks, 128)
        g = x0p[np.where(src >= 0, src, N_NODES)]
        e1_stream[r] = g.transpose(1, 0, 2).reshape(P, -1).astype(bf16)

    return dict(sched_e1=sched_e1, sched_e2=sched_e2, sched_n=sched_n,
                e1_stream=e1_stream, dims=dims)


def _build(pre):
    import concourse.bacc as bacc
    import concourse.mybir as mybir
    import concourse.tile as tile

    dt = mybir.dt
    dims = pre["dims"]
    nc = bacc.Bacc("TRN2", target_bir_lowering=False, debug=False,
                   num_devices=NCORES)

    s_e1, s_e2, s_n = pre["sched_e1"], pre["sched_e2"], pre["sched_n"]

    def din(name, shape, dtyp):
        return nc.dram_tensor(name, shape, dtyp, kind="ExternalInput")

    e1_g = din("e1_g", [P, s_e1.n_blocks * 16], dt.bfloat16)
    e1_seg = din("e1_seg", [P, max(s_e1.n_jobs, 1)], dt.float32)
    e1_w = din("e1_w", [P, s_e1.n_blocks], dt.float32)
    n1_idx = din("n1_idx", [P, s_n.idx_cols], dt.int16)
    n1_seg = din("n1_seg", [P, max(s_n.n_jobs, 1)], dt.float32)
    n1_w = din("n1_w", [P, s_n.n_blocks], dt.float32)
    e2_idx = din("e2_idx", [P, s_e2.idx_cols], dt.int16)
    e2_seg = din("e2_seg", [P, max(s_e2.n_jobs, 1)], dt.float32)
    e2_w = din("e2_w", [P, s_e2.n_blocks], dt.float32)

    w_in = {k: din(k, [kd, HID], dt.bfloat16)
            for k, kd in (("w0_1", 16), ("w1_1", HID), ("w0_2", HID),
                          ("w1_2", HID))}
    b_in = {k: din(k, [P, 1], dt.float32)
            for k in ("b1_1", "b0_1", "b1_2", "b0_2")}
    lin_w = din("lin_w", [P, 1], dt.float32)
    lin_b = din("lin_b", [1, 1], dt.float32)
    iota_in = din("iota", [P, WIN], dt.float32)
    ident_in = din("ident", [P, P], dt.bfloat16)
    out_t = nc.dram_tensor("out", [1, 1], dt.float32, kind="ExternalOutput")

    def dint(name, shape, shared=False):
        return nc.dram_tensor(name, shape, dt.bfloat16, kind="Internal",
                              addr_space="Shared" if shared else "Local")

    x1l1_loc = dint("x1l1_loc", [dims["edge_pad"], HID])
    x1l1_full = dint("x1l1_full", [dims["edge_tab"], HID], True)
    x0p_loc = dint("x0p_loc", [dims["node_pad"], HID])
    x0p_full = dint("x0p_full", [dims["node_tab"], HID], True)
    x1l2_loc = dint("x1l2_loc", [dims["edge_pad"], HID])
    x1l2_full = dint("x1l2_full", [dims["edge_tab"], HID], True)
    armax_in = nc.dram_tensor("armax_in", [P, 1], dt.float32, kind="Internal")
    armax_out = nc.dram_tensor("armax_out", [P, 1], dt.float32,
                               kind="Internal", addr_space="Shared")

    import os as _os0
    GP_BUFS = int(_os0.environ.get("GP_BUFS", "6"))
    SP_BUFS = int(_os0.environ.get("SP_BUFS", "4"))
    with tile.TileContext(nc) as tc:
        with tc.tile_pool(name="const", bufs=1) as cp, \
             tc.tile_pool(name="meta", bufs=int(_os0.environ.get("MP_BUFS", "2"))) as mp, \
             tc.tile_pool(name="gt", bufs=GP_BUFS) as gp, \
             tc.tile_pool(name="st", bufs=SP_BUFS) as sp, \
             tc.tile_pool(name="fl", bufs=2) as fp, \
             tc.tile_pool(name="psw", bufs=1, space="PSUM") as pw, \
             tc.tile_pool(name="psm", bufs=2, space="PSUM") as pm:

            iota_t = cp.tile([P, WIN], dt.float32)
            ident_t = cp.tile([P, P], dt.bfloat16)
            nc.sync.dma_start(iota_t[:], iota_in[:])
            nc.sync.dma_start(ident_t[:], ident_in[:])
            wts, bias = {}, {}
            for k, hnd in w_in.items():
                t = cp.tile(list(hnd.shape), dt.bfloat16, tag=k)
                nc.sync.dma_start(t[:], hnd[:])
                wts[k] = t
            for k, hnd in b_in.items():
                t = cp.tile([P, 1], dt.float32, tag=k)
                nc.sync.dma_start(t[:], hnd[:])
                bias[k] = t
            linw_t = cp.tile([P, 1], dt.float32)
            nc.sync.dma_start(linw_t[:], lin_w[:])
            linb_t = cp.tile([1, 1], dt.float32)
            nc.sync.dma_start(linb_t[:], lin_b[:])
            maxacc = cp.tile([P, WIN], dt.bfloat16)
            nc.vector.memset(maxacc[:], -1.0)

            def run_agg(sched, seg_d, w_d, kdim, wkey, bkey, table, out_loc,
                        idx_d=None, stream_d=None, maxpool=False):
                import os as _os
                LHSW = int(_os.environ.get("LHSW", "0"))
                if LHSW and sched.gathered:
                    kdim = LHSW
                seg_t = mp.tile([P, max(sched.n_jobs, 1)], dt.float32,
                                tag="seg")
                w_t = mp.tile([P, sched.n_blocks], dt.float32, tag="w")
                nc.sync.dma_start(seg_t[:], seg_d[:])
                nc.sync.dma_start(w_t[:], w_d[:])

                import os as _os
                NOGATH = int(_os.environ.get("NOGATH", "0"))
                NOJOBS = int(_os.environ.get("NOJOBS", "0"))
                blk_slice = {}
                if sched.gathered:
                    idx_t = mp.tile([P, sched.idx_cols], dt.int16, tag="idx")
                    nc.sync.dma_start(idx_t[:], idx_d[:])
                    tab_rows = table.shape[0]
                    for k, (b, slot_off, ni) in enumerate(sched.insts):
                        g = gp.tile([P, (NIMAX // P) * HID], dt.bfloat16,
                                    tag="g")
                        off = sched.inst_idx_off[k]
                        if not NOGATH:
                            # multi-packet descriptors: ~10% faster at the
                            # 256B/row descriptor size of these gathers
                            nc.gpsimd.dma_gather(
                                g[:, :(ni // P) * HID].rearrange(
                                    "p (n f) -> p n f", f=HID),
                                table[b * BUCK:min((b + 1) * BUCK, tab_rows), :],
                                idx_t[:, off:off + ni // 16],
                                ni, ni, HID, single_packet=False)
                        else:
                            nc.vector.memset(g[:1, :1], 0.0)
                        for cb in range(ni // P):
                            blk_slice[slot_off // P + cb] = (g, cb * HID, HID)
                else:
                    SLAB = 32
                    for sl in range(-(-sched.n_blocks // SLAB)):
                        b0 = sl * SLAB
                        nb = min(SLAB, sched.n_blocks - b0)
                        g = gp.tile([P, SLAB * 16], dt.bfloat16, tag="g")
                        nc.sync.dma_start(g[:, :nb * 16],
                                          stream_d[:, b0 * 16:(b0 + nb) * 16])
                        for cb in range(nb):
                            blk_slice[b0 + cb] = (g, cb * 16, 16)

                win_tiles = {}

                def flush(wn):
                    psum1 = win_tiles.pop(wn)
                    aggt = fp.tile([kdim, WIN], dt.bfloat16, tag="aggt")
                    nc.vector.tensor_copy(aggt[:], psum1[:])
                    psum2 = pm.tile([P, WIN], dt.float32, tag="m",
                                    space="PSUM")
                    nc.tensor.matmul(psum2[:], lhsT=wts[wkey][:kdim, :],
                                     rhs=aggt[:], start=True, stop=True)
                    xt = fp.tile([P, WIN], dt.bfloat16, tag="xt")
                    nc.scalar.activation(xt[:], psum2[:],
                                         mybir.ActivationFunctionType.Sigmoid,
                                         bias=bias[bkey][:, :1], scale=1.0)
                    if maxpool:
                        nv = min(WIN, sched.n_seg_loc - wn * WIN)
                        nc.vector.tensor_tensor(
                            out=maxacc[:, :nv], in0=maxacc[:, :nv],
                            in1=xt[:, :nv], op=mybir.AluOpType.max)
                    else:
                        nq = WIN // P
                        rowt = fp.tile([P, WIN], dt.bfloat16, tag="rowt")
                        for q in range(nq):
                            pt = pw.tile([P, P], dt.bfloat16, tag="tp",
                                         space="PSUM")
                            nc.tensor.transpose(pt[:],
                                                xt[:, q * P:(q + 1) * P],
                                                ident_t[:])
                            nc.vector.tensor_copy(rowt[:, q * P:(q + 1) * P],
                                                  pt[:])
                        nc.sync.dma_start(
                            out_loc[wn * WIN:(wn + 1) * WIN, :].rearrange(
                                "(q p) f -> p q f", p=P),
                            rowt[:].rearrange("p (q f) -> p q f", q=nq))

                TS_MEMSET = int(_os.environ.get("TS_MEMSET", "0"))
                MM_FIXED = int(_os.environ.get("MM_FIXED", "0"))
                MM_SS = int(_os.environ.get("MM_SS", "0"))
                variant = sched.gathered
                for j, (blk, wn, c0, c1) in enumerate(sched.jobs):
                    if NOJOBS:
                        break
                    g, goff, gw = blk_slice[blk]
                    span = c1 - c0
                    s_t = sp.tile([P, WIN], dt.bfloat16, tag="s")
                    if variant and TS_MEMSET:
                        nc.vector.memset(s_t[:, :span], 0.5)
                    else:
                        nc.vector.tensor_scalar(
                            out=s_t[:, :span], in0=iota_t[:, c0:c1],
                            scalar1=seg_t[:, j:j + 1],
                            scalar2=w_t[:, blk:blk + 1],
                            op0=mybir.AluOpType.is_equal,
                            op1=mybir.AluOpType.mult)
                    if wn not in win_tiles:
                        pt = pw.tile([kdim, WIN], dt.float32,
                                     tag=f"win{wn % SECW}", space="PSUM")
                        nc.vector.memset(pt[:], 0.0)
                        win_tiles[wn] = pt
                    mmc0, mmc1 = (0, span) if (variant and MM_FIXED) else (c0, c1)
                    if variant and MM_SS:
                        nc.tensor.matmul(
                            win_tiles[wn][:, mmc0:mmc1],
                            lhsT=g[:, goff:goff + kdim],
                            rhs=s_t[:, :span], start=True, stop=True,
                            skip_group_check=True)
                    else:
                        nc.tensor.matmul(
                            win_tiles[wn][:, mmc0:mmc1],
                            lhsT=g[:, goff:goff + kdim],
                            rhs=s_t[:, :span], start=False,
                            stop=(sched.win_last[wn] == j),
                            skip_group_check=True)
                    if sched.win_last[wn] == j:
                        flush(wn)

            import os
            PH = int(os.environ.get("PHASES", "4"))
            NOCOLL = int(os.environ.get("NOCOLL", "0"))
            rg = [list(range(NCORES))]
            if PH >= 1:
                run_agg(s_e1, e1_seg, e1_w, 16, "w0_1", "b1_1", None, x1l1_loc,
                        stream_d=e1_g)
            if PH >= 2:
                if not NOCOLL:
                    nc.gpsimd.collective_compute(
                        "AllGather", mybir.AluOpType.bypass, replica_groups=rg,
                        ins=[x1l1_loc[:]], outs=[x1l1_full[:]])
                run_agg(s_n, n1_seg, n1_w, HID, "w1_1", "b0_1", x1l1_full,
                        x0p_loc, idx_d=n1_idx)
            if PH >= 3:
                if not NOCOLL:
                    nc.gpsimd.collective_compute(
                        "AllGather", mybir.AluOpType.bypass, replica_groups=rg,
                        ins=[x0p_loc[:]], outs=[x0p_full[:]])
                run_agg(s_e2, e2_seg, e2_w, HID, "w0_2", "b1_2", x0p_full,
                        x1l2_loc, idx_d=e2_idx)
            if PH >= 4:
                if not NOCOLL:
                    nc.gpsimd.collective_compute(
                        "AllGather", mybir.AluOpType.bypass, replica_groups=rg,
                        ins=[x1l2_loc[:]], outs=[x1l2_full[:]])
                run_agg(s_n, n1_seg, n1_w, HID, "w1_2", "b0_2", x1l2_full,
                        None, idx_d=n1_idx, maxpool=True)

            mx = fp.tile([P, 1], dt.float32, tag="mx")
            nc.vector.reduce_max(out=mx[:], in_=maxacc[:],
                                 axis=mybir.AxisListType.X)
            nc.sync.dma_start(armax_in[:], mx[:])
            nc.gpsimd.collective_compute(
                "AllReduce", mybir.AluOpType.max, replica_groups=rg,
                ins=[armax_in[:]], outs=[armax_out[:]])
            mx2 = fp.tile([P, 1], dt.float32, tag="mx2")
            nc.sync.dma_start(mx2[:], armax_out[:])
            prod = fp.tile([P, 1], dt.float32, tag="prod")
            nc.vector.tensor_mul(prod[:], mx2[:], linw_t[:])
            ones = cp.tile([P, 1], dt.float32, tag="ones")
            nc.vector.memset(ones[:], 1.0)
            psf = pw.tile([1, 1], dt.float32, tag="tp", space="PSUM")
            nc.tensor.matmul(psf[:], lhsT=prod[:], rhs=ones[:],
                             start=True, stop=True)
            res = fp.tile([1, 1], dt.float32, tag="res")
            nc.scalar.activation(res[:], psf[:],
                                 mybir.ActivationFunctionType.Identity,
                                 bias=linb_t[:, :1], scale=1.0)
            nc.sync.dma_start(out_t[:], res[:])

    nc.compile()
    return nc


def make_in_maps(pre, inputs):
    s_e1, s_e2, s_n = pre["sched_e1"], pre["sched_e2"], pre["sched_n"]
    iota = np.broadcast_to(np.arange(WIN, dtype=f32), (P, WIN)).copy()
    ident = np.eye(P, dtype=bf16)

    def b_t(x):
        return np.asarray(x).astype(f32).reshape(HID, 1)

    w0_1 = np.zeros((16, HID), bf16)
    w0_1[:IN_CH] = np.asarray(inputs["w0_l1"]).astype(bf16)
    in_maps = []
    for r in range(NCORES):
        in_maps.append(dict(
            e1_g=pre["e1_stream"][r],
            e1_seg=np.ascontiguousarray(s_e1.seg_arr[r]),
            e1_w=np.ascontiguousarray(s_e1.w_arr[r]),
            n1_idx=np.ascontiguousarray(s_n.idx_arr[r]),
            n1_seg=np.ascontiguousarray(s_n.seg_arr[r]),
            n1_w=np.ascontiguousarray(s_n.w_arr[r]),
            e2_idx=np.ascontiguousarray(s_e2.idx_arr[r]),
            e2_seg=np.ascontiguousarray(s_e2.seg_arr[r]),
            e2_w=np.ascontiguousarray(s_e2.w_arr[r]),
            w0_1=w0_1,
            w1_1=np.asarray(inputs["w1_l1"]).astype(bf16),
            w0_2=np.asarray(inputs["w0_l2"]).astype(bf16),
            w1_2=np.asarray(inputs["w1_l2"]).astype(bf16),
            b1_1=b_t(inputs["b1_l1"]), b0_1=b_t(inputs["b0_l1"]),
            b1_2=b_t(inputs["b1_l2"]), b0_2=b_t(inputs["b0_l2"]),
            lin_w=np.asarray(inputs["lin_w"]).astype(f32).reshape(HID, 1),
            lin_b=np.asarray(inputs["lin_b"]).astype(f32).reshape(1, 1),
            iota=iota, ident=ident,
        ))
    return in_maps


def kernel(**inputs):
    pre = _preprocess(inputs)
    nc = _build(pre)
    in_maps = make_in_maps(pre, inputs)
    from concourse.bass_utils import run_bass_kernel_spmd
    res = run_bass_kernel_spmd(nc, in_maps, core_ids=list(range(NCORES)))
    out = res.results[0]["out"].reshape(1).astype(f32)
    return out

